# revision 13
# baseline (speedup 1.0000x reference)
# Trainium2 Bass kernel for: ConvTranspose2d(64->128, k=4, stride=1) -> spatial
# mean -> +biases -> 10*logsumexp over channels.
#
# Math: with full (K-1) output padding, the mean over the ENTIRE conv-transpose
# output spatial extent sees every input pixel through all K*K taps, so
#   pooled[n,co] = (sum_hw x[n,ci,hw]) @ (sum_kk w[ci,co,kk]) / (Ho*Wo) + cb + eb
# exactly. The conv collapses to a spatial sum + a (Cin x Cout) matmul.
#
# Sharding: data-parallel over batch N=32 across 8 cores (4 batches/core).
# The (Cin,Cout) tap-sum of the replicated weight is precomputed on the host
# (param preprocessing, like weight repacking), so each core only streams its
# 4 MiB x-slice plus ~70 KiB of params.
#
# Per-core dataflow (trace-driven, see test.py profiling):
# - x arrives as [256, 4096] (row = (n,ci)); the stream is HBM-bound at
#   ~358 GB/s, so everything else is scheduled around its completion order.
# - Column chunks ride both HWDGE rings: the ACT ring (qScalarDynamicHW)
#   carries 2x800 cols per row block, reduced on ACT via activation(Copy,
#   accum_out); the SP ring (qSyncDynamicHW) carries a tapered 736/648/580/532
#   split, reduced on DVE. The rings drain at equal packet rates, so the
#   lighter ACT ring finishes ~2.5us early and ACT is idle before the stream
#   ends; the DVE taper solves the keep-up recurrence (reduce(c_k) <=
#   landing-gap(c_{k+1})) so only a 532-col reduce trails the last byte.
# - Per (row-block, engine) group, partials combine DIRECTLY into the masked
#   f32r lhsT columns (half-partition reduces on DVE, Copy+accum on ACT), and
#   a single-pass fp32r matmul accumulates the group into PSUM on top of an
#   early bias matmul. Only the last group's short chain trails the stream.
# - Params ride the GpSimd SWDGE queue (issued first, fresh semaphores, never
#   blocking the HWDGE rings); zeros for the masked tiles are DMA'd because
#   no engine can memset f32r.
# - exp-accumulate + log + 10x on ACT, one table set (Exp+Ln+Copy) preloaded
#   at kernel start so no ACT_TABLE_LOAD lands anywhere; y leaves on the idle
#   SP ring.

import os

import numpy as np

import concourse.bacc as bacc
import concourse.bass as bass
import concourse.mybir as mybir
import concourse.tile as tile
from concourse.bass_utils import run_bass_kernel_spmd
from concourse.hw_specs import get_activation_tables

N, CIN, COUT, K, H, W = 32, 64, 128, 4, 64, 64
NCORES = 8
NLOC = N // NCORES          # 4 batches per core
HW = H * W                  # 4096
ROWS = NLOC * CIN           # 256 rows (n,ci) per core
RBLK = ROWS // 128          # 2 row blocks of 128 partitions
ACOLS = [800, 800]                    # ACT-ring / ACT-reduced chunks
DCOLS = [736, 648, 580, 532]          # SP-ring / DVE-reduced, tapered
assert sum(ACOLS) + sum(DCOLS) == HW
NCH = len(ACOLS) + len(DCOLS)         # 6 chunks per row block
NGRP = RBLK * 2                       # (row block, engine) masked groups
SCALE = 1.0 / float((H + K - 1) * (W + K - 1))   # 1/4489

F32 = mybir.dt.float32
F32R = mybir.dt.float32r

_CACHE: dict = {}


def _build_module() -> bacc.Bacc:
    nc = bacc.Bacc("TRN2", target_bir_lowering=False, enable_partition_id=False)

    x_d = nc.dram_tensor("xc", [ROWS, HW], F32, kind="ExternalInput").ap()
    w_d = nc.dram_tensor("wsum", [128, COUT], F32R, kind="ExternalInput").ap()
    bs_d = nc.dram_tensor("bs", [2, COUT], F32R, kind="ExternalInput").ap()
    z_d = nc.dram_tensor("zm", [128, NGRP * NLOC], F32R, kind="ExternalInput").ap()
    o_d = nc.dram_tensor("ones", [2, NLOC], F32R, kind="ExternalInput").ap()
    y_d = nc.dram_tensor("y", [NLOC, 1], F32, kind="ExternalOutput").ap()

    with tile.TileContext(nc) as tc:
        with (
            tc.tile_pool(name="xpool", bufs=1) as xpool,
            tc.tile_pool(name="spool", bufs=2) as spool,
            tc.tile_pool(name="small", bufs=1) as small,
            tc.tile_pool(name="psum", bufs=1, space="PSUM") as psum_pool,
        ):
            # preload the one ACT table set that covers Exp, Ln AND Copy
            # ("natural_log_exp_and_others") so no ACT_TABLE_LOAD is inserted
            # anywhere in the chain.
            act_tables = get_activation_tables(nc.m.arch)
            set_id = next(
                i
                for i, (_, funcs) in enumerate(act_tables.items())
                if mybir.ActivationFunctionType.Exp in funcs
                and mybir.ActivationFunctionType.Ln in funcs
                and mybir.ActivationFunctionType.Copy in funcs
            )
            nc.scalar.add_instruction(
                mybir.InstLoadActFuncSet(
                    name=nc.get_next_instruction_name(), act_func_set_id=set_id
                )
            )

            parts = small.tile([128, RBLK * NCH], F32)
            wdup = small.tile([128, COUT], F32R)
            biasrows = small.tile([2, COUT], F32R)
            onesb = small.tile([2, NLOC], F32R)
            s2m = small.tile([128, NGRP * NLOC], F32R)
            scratch = [
                spool.tile([128, max(ACOLS)], F32, name=f"scratch{j}")
                for j in range(2)
            ]

            # params on the SWDGE queue, first (fresh sems, idle engine)
            nc.gpsimd.dma_start(out=s2m, in_=z_d)
            nc.gpsimd.dma_start(out=wdup, in_=w_d)
            nc.gpsimd.dma_start(out=biasrows, in_=bs_d)
            nc.gpsimd.dma_start(out=onesb, in_=o_d)

            # ---- x chunk DMAs, row-block-major on each ring ----
            # column layout per row block: [A0, A1, D0, D1, D2, D3]
            offs = []
            off = 0
            for w_ in ACOLS + DCOLS:
                offs.append((off, w_))
                off += w_
            xts = {}
            for rb in range(RBLK):
                for c, (o, w_) in enumerate(offs):
                    xt = xpool.tile([128, w_], F32, tag=f"xt{rb}_{c}")
                    eng = nc.scalar if c < len(ACOLS) else nc.sync
                    eng.dma_start(
                        out=xt, in_=x_d[rb * 128 : (rb + 1) * 128, o : o + w_]
                    )
                    xts[(rb, c)] = xt

            # ---- early bias matmul opens the PSUM accumulation group ----
            pooled = psum_pool.tile([NLOC, COUT], F32, space="PSUM")
            nc.tensor.matmul(
                out=pooled, lhsT=onesb, rhs=biasrows, start=True, stop=False
            )

            # ---- per-chunk partial sums + per-(rb, engine) group matmuls ----
            # masked group g occupies s2m cols [4g, 4g+4): within it, column
            # 2rb gets the top-half sums, 2rb+1 the bottom-half; the other
            # row block's columns stay zero so accumulation is exact.
            with nc.allow_low_precision(
                reason="f32r combine outputs are 32-bit storage; only the PE "
                "multiply rounds, and rel-err budget is 2e-2"
            ):
                for rb in range(RBLK):
                    pc = rb * NCH
                    for c in range(NCH):
                        xt = xts[(rb, c)]
                        col = pc + c
                        if c < len(ACOLS):
                            nc.scalar.activation(
                                out=scratch[c % 2][:, 0 : xt.shape[1]],
                                in_=xt,
                                func=mybir.ActivationFunctionType.Copy,
                                accum_out=parts[:, col : col + 1],
                            )
                        else:
                            nc.vector.reduce_sum(
                                out=parts[:, col : col + 1],
                                in_=xt,
                                axis=mybir.AxisListType.X,
                            )
                    # ACT group combine (2 partials -> masked f32r halves)
                    ga = (rb * 2) * NLOC
                    nc.scalar.activation(
                        out=scratch[0][0:64, 0:2],
                        in_=parts[0:64, pc : pc + 2],
                        func=mybir.ActivationFunctionType.Copy,
                        accum_out=s2m[0:64, ga + 2 * rb : ga + 2 * rb + 1],
                    )
                    nc.scalar.activation(
                        out=scratch[0][64:128, 0:2],
                        in_=parts[64:128, pc : pc + 2],
                        func=mybir.ActivationFunctionType.Copy,
                        accum_out=s2m[64:128, ga + 2 * rb + 1 : ga + 2 * rb + 2],
                    )
                    nc.tensor.matmul(
                        out=pooled,
                        lhsT=s2m[:, ga : ga + NLOC],
                        rhs=wdup,
                        start=False,
                        stop=False,
                        skip_group_check=True,
                    )
                    # DVE group combine (4 partials -> masked f32r halves)
                    gd = (rb * 2 + 1) * NLOC
                    nc.vector.reduce_sum(
                        out=s2m[0:64, gd + 2 * rb : gd + 2 * rb + 1],
                        in_=parts[0:64, pc + 2 : pc + NCH],
                        axis=mybir.AxisListType.X,
                    )
                    nc.vector.reduce_sum(
                        out=s2m[64:128, gd + 2 * rb + 1 : gd + 2 * rb + 2],
                        in_=parts[64:128, pc + 2 : pc + NCH],
                        axis=mybir.AxisListType.X,
                    )
                    nc.tensor.matmul(
                        out=pooled,
                        lhsT=s2m[:, gd : gd + NLOC],
                        rhs=wdup,
                        start=False,
                        stop=(rb == RBLK - 1),
                        skip_group_check=True,
                    )

            # ---- 10 * log(sum_co exp(pooled)) ----
            expt = small.tile([NLOC, COUT], F32)
            sume = small.tile([NLOC, 1], F32)
            nc.scalar.activation(
                out=expt,
                in_=pooled,
                func=mybir.ActivationFunctionType.Exp,
                accum_out=sume,
            )
            logv = small.tile([NLOC, 1], F32)
            nc.scalar.activation(
                out=logv, in_=sume, func=mybir.ActivationFunctionType.Ln
            )
            outv = small.tile([NLOC, 1], F32)
            nc.scalar.mul(out=outv, in_=logv, mul=10.0)
            nc.sync.dma_start(out=y_d, in_=outv)

    nc.compile()
    return nc


def kernel(x, weight, conv_bias, extra_bias):
    x = np.ascontiguousarray(np.asarray(x, dtype=np.float32))
    weight = np.ascontiguousarray(np.asarray(weight, dtype=np.float32))
    conv_bias = np.ascontiguousarray(np.asarray(conv_bias, dtype=np.float32))
    extra_bias = np.ascontiguousarray(np.asarray(extra_bias, dtype=np.float32))
    assert x.shape == (N, CIN, H, W), x.shape
    assert weight.shape == (CIN, COUT, K, K), weight.shape

    if "nc" not in _CACHE:
        _CACHE["nc"] = _build_module()
    nc = _CACHE["nc"]

    # host-side param preprocessing: scaled tap-sum, duplicated onto both
    # partition halves so each batch contracts against its own half.
    ws = (weight.reshape(CIN, COUT, K * K).sum(axis=2) * SCALE).astype(np.float32)
    wdup = np.ascontiguousarray(np.vstack([ws, ws]))  # (128, COUT)
    bs2 = np.ascontiguousarray(
        np.stack([conv_bias, extra_bias], axis=0)
    )  # (2, COUT)
    zm = np.zeros((128, NGRP * NLOC), dtype=np.float32)
    ones = np.ones((2, NLOC), dtype=np.float32)
    in_maps = []
    for c in range(NCORES):
        xc = x[c * NLOC : (c + 1) * NLOC].reshape(ROWS, HW)
        in_maps.append(
            {"xc": xc, "wsum": wdup, "bs": bs2, "zm": zm, "ones": ones}
        )

    trace = os.environ.get("BASS_KERNEL_TRACE") == "1"
    res = run_bass_kernel_spmd(
        nc, in_maps, core_ids=list(range(NCORES)), trace=trace
    )
    _CACHE["last_result"] = res
    return np.concatenate([r["y"] for r in res.results], axis=0)


# revision 14
# speedup vs baseline: 1.0223x; 1.0223x over previous
# Trainium2 Bass kernel for: ConvTranspose2d(64->128, k=4, stride=1) -> spatial
# mean -> +biases -> 10*logsumexp over channels.
#
# Math: with full (K-1) output padding, the mean over the ENTIRE conv-transpose
# output spatial extent sees every input pixel through all K*K taps, so
#   pooled[n,co] = (sum_hw x[n,ci,hw]) @ (sum_kk w[ci,co,kk]) / (Ho*Wo) + cb + eb
# exactly. The conv collapses to a spatial sum + a (Cin x Cout) matmul.
#
# Sharding: data-parallel over batch N=32 across 8 cores (4 batches/core).
# The (Cin,Cout) tap-sum of the replicated weight is precomputed on the host
# (param preprocessing, like weight repacking), so each core only streams its
# 4 MiB x-slice plus one 68 KiB packed param tensor.
#
# Per-core dataflow (trace-driven, see test.py profiling):
# - x arrives as [256, 4096] (row = (n,ci)); the stream is HBM-bound at
#   ~350 GB/s, so everything is scheduled around its completion order, and
#   descriptors are kept >=2.5 KiB (smaller ones measurably cost bandwidth).
# - Chunks ride both HWDGE rings: the ACT ring (qScalarDynamicHW) carries
#   [1152, 640] cols per row block, reduced on ACT via activation(Copy,
#   accum_out); the SP ring (qSyncDynamicHW) carries [896, 768, 640], reduced
#   on DVE. Rings drain at equal packet rates, so the lighter ACT ring
#   finishes ~1.5us early and ACT is free before the stream ends; the DVE
#   taper solves the keep-up recurrence so only a ~0.8us reduce+combine chain
#   trails the last byte.
# - TRANSPOSED matmul orientation: pooledT[co, n] = wdup.T @ s2m with the
#   pre-duplicated weight sums as the fp32r stationary and the zero-masked
#   per-row-block sums (written DIRECTLY by half-partition DVE reduces, f32r)
#   as the 4-column moving operand. Biases fold into Exp's per-partition bias
#   port, so there is NO bias matmul and NO accumulator read in the tail; the
#   channel sum of exp is a single-pass bf16 ones-matmul into PSUM.
# - One packed param DMA (weights | zero masks | bias vector) rides the SP
#   ring first with a fresh semaphore; ACT-table set (Exp+Ln+Copy) preloaded
#   so no ACT_TABLE_LOAD lands anywhere; y leaves on the idle SP ring.

import os

import numpy as np

import concourse.bacc as bacc
import concourse.bass as bass
import concourse.mybir as mybir
import concourse.tile as tile
from concourse.bass_utils import run_bass_kernel_spmd
from concourse.hw_specs import get_activation_tables

N, CIN, COUT, K, H, W = 32, 64, 128, 4, 64, 64
NCORES = 8
NLOC = N // NCORES          # 4 batches per core
HW = H * W                  # 4096
ROWS = NLOC * CIN           # 256 rows (n,ci) per core
RBLK = ROWS // 128          # 2 row blocks of 128 partitions
ACOLS = [1152, 640]         # ACT-ring / ACT-reduced chunks (per row block)
DCOLS = [896, 768, 640]     # SP-ring / DVE-reduced, tapered
assert sum(ACOLS) + sum(DCOLS) == HW
NCH = len(ACOLS) + len(DCOLS)         # 5 chunks per row block
SCALE = 1.0 / float((H + K - 1) * (W + K - 1))   # 1/4489

# packed param layout (one [128, PCOLS] f32r tensor):
#   [0:COUT)        wdup   - scaled weight tap-sums, duplicated on both halves
#   [COUT:COUT+8)   s2m    - zero-initialized masked moving operands (2 groups)
#   [COUT+8]        bvec   - conv_bias + extra_bias (read as f32)
PCOLS = COUT + RBLK * NLOC + 1

F32 = mybir.dt.float32
F32R = mybir.dt.float32r
BF16 = mybir.dt.bfloat16

_CACHE: dict = {}


def _build_module() -> bacc.Bacc:
    nc = bacc.Bacc("TRN2", target_bir_lowering=False, enable_partition_id=False)

    x_d = nc.dram_tensor("xc", [ROWS, HW], F32, kind="ExternalInput").ap()
    p_d = nc.dram_tensor("pk", [128, PCOLS], F32R, kind="ExternalInput").ap()
    y_d = nc.dram_tensor("y", [1, NLOC], F32, kind="ExternalOutput").ap()

    with tile.TileContext(nc) as tc:
        with (
            tc.tile_pool(name="xpool", bufs=1) as xpool,
            tc.tile_pool(name="spool", bufs=2) as spool,
            tc.tile_pool(name="small", bufs=1) as small,
            tc.tile_pool(name="psum", bufs=1, space="PSUM") as psum_pool,
        ):
            # preload the one ACT table set that covers Exp, Ln AND Copy
            # ("natural_log_exp_and_others") so no ACT_TABLE_LOAD is inserted
            # anywhere in the chain.
            act_tables = get_activation_tables(nc.m.arch)
            set_id = next(
                i
                for i, (_, funcs) in enumerate(act_tables.items())
                if mybir.ActivationFunctionType.Exp in funcs
                and mybir.ActivationFunctionType.Ln in funcs
                and mybir.ActivationFunctionType.Copy in funcs
            )
            nc.scalar.add_instruction(
                mybir.InstLoadActFuncSet(
                    name=nc.get_next_instruction_name(), act_func_set_id=set_id
                )
            )

            parts = small.tile([128, RBLK * NCH], F32)
            param = small.tile([128, PCOLS], F32R)
            onesb = small.tile([128, 1], BF16)
            scratch = [
                spool.tile([128, max(ACOLS)], F32, name=f"scratch{j}")
                for j in range(2)
            ]

            wdup = param[:, 0:COUT]
            s2m = param[:, COUT : COUT + RBLK * NLOC]
            bvec = param.bitcast(F32)[:, COUT + RBLK * NLOC : PCOLS]

            # packed params ride the SP ring first (fresh semaphore, lands
            # within ~0.5us of the stream start, needed only mid-kernel)
            nc.sync.dma_start(out=param, in_=p_d)
            nc.vector.memset(onesb, 1.0)

            # ---- x chunk DMAs, row-block-major on each ring ----
            # column layout per row block: [A0, A1, D0, D1, D2]
            offs = []
            off = 0
            for w_ in ACOLS + DCOLS:
                offs.append((off, w_))
                off += w_
            xts = {}
            for rb in range(RBLK):
                for c, (o, w_) in enumerate(offs):
                    xt = xpool.tile([128, w_], F32, tag=f"xt{rb}_{c}")
                    eng = nc.scalar if c < len(ACOLS) else nc.sync
                    eng.dma_start(
                        out=xt, in_=x_d[rb * 128 : (rb + 1) * 128, o : o + w_]
                    )
                    xts[(rb, c)] = xt

            pooledT = psum_pool.tile([128, NLOC], F32, space="PSUM")

            # ---- per-chunk partial sums + per-row-block masked matmul ----
            # masked group rb occupies s2m cols [4rb, 4rb+4): col 2rb carries
            # the top-half sums, 2rb+1 the bottom-half, others stay zero, so
            # pooledT[co, n] accumulates exactly each batch's contraction.
            with nc.allow_low_precision(
                reason="f32r combine outputs are 32-bit storage; only the PE "
                "multiply rounds, and rel-err budget is 2e-2"
            ):
                for rb in range(RBLK):
                    pc = rb * NCH
                    for c in range(NCH):
                        xt = xts[(rb, c)]
                        col = pc + c
                        if c < len(ACOLS):
                            nc.scalar.activation(
                                out=scratch[c % 2][:, 0 : xt.shape[1]],
                                in_=xt,
                                func=mybir.ActivationFunctionType.Copy,
                                accum_out=parts[:, col : col + 1],
                            )
                        else:
                            nc.vector.reduce_sum(
                                out=parts[:, col : col + 1],
                                in_=xt,
                                axis=mybir.AxisListType.X,
                            )
                    # combine all 5 partials straight into the masked f32r
                    # halves (half-partition reduces cost only ~170ns each)
                    g = COUT * 0 + 4 * rb  # s2m-local group base
                    nc.vector.reduce_sum(
                        out=s2m[0:64, g + 2 * rb : g + 2 * rb + 1],
                        in_=parts[0:64, pc : pc + NCH],
                        axis=mybir.AxisListType.X,
                    )
                    nc.vector.reduce_sum(
                        out=s2m[64:128, g + 2 * rb + 1 : g + 2 * rb + 2],
                        in_=parts[64:128, pc : pc + NCH],
                        axis=mybir.AxisListType.X,
                    )
                    nc.tensor.matmul(
                        out=pooledT,
                        lhsT=wdup,
                        rhs=s2m[:, g : g + NLOC],
                        start=(rb == 0),
                        stop=(rb == RBLK - 1),
                        skip_group_check=True,
                    )

            # ---- 10 * log(sum_co exp(pooledT + bias)) ----
            exptT = small.tile([128, NLOC], BF16)
            nc.scalar.activation(
                out=exptT,
                in_=pooledT,
                func=mybir.ActivationFunctionType.Exp,
                bias=bvec,
            )
            sumeT = psum_pool.tile([1, NLOC], F32, space="PSUM")
            nc.tensor.matmul(out=sumeT, lhsT=onesb, rhs=exptT, start=True, stop=True)
            logv = small.tile([1, NLOC], F32)
            nc.scalar.activation(
                out=logv, in_=sumeT, func=mybir.ActivationFunctionType.Ln
            )
            outv = small.tile([1, NLOC], F32)
            nc.scalar.mul(out=outv, in_=logv, mul=10.0)
            nc.sync.dma_start(out=y_d, in_=outv)

    nc.compile()
    return nc


def kernel(x, weight, conv_bias, extra_bias):
    x = np.ascontiguousarray(np.asarray(x, dtype=np.float32))
    weight = np.ascontiguousarray(np.asarray(weight, dtype=np.float32))
    conv_bias = np.ascontiguousarray(np.asarray(conv_bias, dtype=np.float32))
    extra_bias = np.ascontiguousarray(np.asarray(extra_bias, dtype=np.float32))
    assert x.shape == (N, CIN, H, W), x.shape
    assert weight.shape == (CIN, COUT, K, K), weight.shape

    if "nc" not in _CACHE:
        _CACHE["nc"] = _build_module()
    nc = _CACHE["nc"]

    # host-side param packing: scaled weight tap-sums duplicated onto both
    # partition halves | zeroed mask groups | summed bias vector.
    ws = (weight.reshape(CIN, COUT, K * K).sum(axis=2) * SCALE).astype(np.float32)
    pk = np.zeros((128, PCOLS), dtype=np.float32)
    pk[0:CIN, 0:COUT] = ws
    pk[CIN:128, 0:COUT] = ws
    pk[:, COUT + RBLK * NLOC] = (conv_bias + extra_bias).astype(np.float32)
    pk = np.ascontiguousarray(pk)
    in_maps = []
    for c in range(NCORES):
        xc = x[c * NLOC : (c + 1) * NLOC].reshape(ROWS, HW)
        in_maps.append({"xc": xc, "pk": pk})

    trace = os.environ.get("BASS_KERNEL_TRACE") == "1"
    res = run_bass_kernel_spmd(
        nc, in_maps, core_ids=list(range(NCORES)), trace=trace
    )
    _CACHE["last_result"] = res
    return np.concatenate(
        [np.asarray(r["y"]).reshape(NLOC, 1) for r in res.results], axis=0
    )
